# revision 50
# baseline (speedup 1.0000x reference)
"""Multi-head attention (B=2, S=2048, D=1024, H=16, causal) on 8 trn2 cores.

Sharding: core c -> batch b = c//4, head group g = c%4 (4 heads of 64 dims):
data parallel over batch, tensor/head parallel within it (W_q/W_k/W_v split
column-wise, W_o row-wise per head group).  Each core computes Q/K/V
projections for its head group over the full sequence, causal flash-style
attention, and the partial output projection A_g @ Wo.T[g_rows, :].  The host
pre-transposes activations/weight slices to fp16, sums the 4 output partials
per batch (the row-parallel unshard), and adds bo.

Device dataflow (per core, fp16 matmuls, fp32 accumulation/softmax):
  inputs stream in 512-column chunks in first-use order so the PE never
           stalls on DMA and stays out of the low p-state
  QT, KT   [256, S] head-dim-major, computed by projection chains
           interleaved into the attention stream as PE gap fillers
  V_store  [S, 4*65] v columns + a ones column per head (so the AV matmul
           also produces the softmax denominators); the 16 V chains are
           themselves fillers (chunk qc's chains run early inside qc's own
           pair-0 stream, each one ahead of the AV tile that consumes it)
  scores^T tiles [128 ki, <=512 qi] -> exp on ACT (scale=1/sqrt(64)) -> E,
           diagonal tiles trimmed to their valid qi range; causal mask
           multiply only on the 128 straddling columns
  A^T      [256, S] accumulated in PSUM via (V|1)^T @ E, normalized by
           reciprocal of the ones-row read straight out of PSUM
           (gpsimd partition broadcast + fused multiply)
  out      [S, 1024] fp16 partial = A_g @ WoT_g, projected in per-chunk
           bursts slipped into the next chunk's attention stream

Scheduling (what got this from ~192us to ~163us):
  - DMA queues: only sync/scalar/gpsimd can issue DMAs.  A sync/scalar
    dispatch occupies the issuing engine until the queue ring (~2 deep)
    frees, and ring descriptors stream concurrently, splitting that
    queue's share of the ~340GB/s aggregate.  So the exp engine (scalar)
    gets exactly two early half-chunk transfers, sync's engine (no
    compute) eats the whole late pile, and gpsimd's never-blocking
    software queue carries weights + mid chunks, with the last chunks
    enqueued mid-loop (GPACED) so they can't steal early bandwidth.
  - The c0 halves are striped across sync+scalar so xq0 and xk0 land
    together (~16us); warm-up matmuls cover the DMA ramp and hold the
    PE's DVFS p-state up.
  - Every pair after the first gets a projection chain (or an out-
    projection burst half in the last chunk) in its t=0 filler slot to
    cover the previous pair's norm latency + exp pipeline refill.
  - The final burst prestarts 6 of 8 units' at0 accumulation in freed
    psum banks before the last norm, and the closing copies/DMAs are
    split across scalar+vector / sync+scalar to shorten the tail.
"""

import numpy as np

import concourse.bacc as bacc
import concourse.mybir as mybir
import concourse.tile as tile
from concourse.bass_utils import run_bass_kernel_spmd

F32 = mybir.dt.float32
BF16 = mybir.dt.float16  # fp16: same PE speed as bf16, 4x the mantissa
NP_BF16 = np.float16

S = 2048        # sequence length
E = 1024        # model dim (contraction for projections)
DG = 256        # head-group dim (4 heads x 64)
DH = 64         # head dim
NH = 4          # heads per core
ET = E // 128   # 8 e-tiles
ST = S // 128   # 16 s-tiles
SC = 512        # sequence chunk (psum free dim)
NSC = S // SC   # 4 chunks
SCALE = 1.0 / np.sqrt(DH)

_CACHED = {}


def _build():
    nc = bacc.Bacc("TRN2", target_bir_lowering=False, debug=False, num_devices=8)

    xqT = nc.dram_tensor("xqT", [E, S], BF16, kind="ExternalInput")
    xkT = nc.dram_tensor("xkT", [E, S], BF16, kind="ExternalInput")
    xvT = nc.dram_tensor("xvT", [E, S], BF16, kind="ExternalInput")
    wqT = nc.dram_tensor("wqT", [E, DG], BF16, kind="ExternalInput")
    wkT = nc.dram_tensor("wkT", [E, DG], BF16, kind="ExternalInput")
    wvT = nc.dram_tensor("wvT", [E, DG], BF16, kind="ExternalInput")
    woT = nc.dram_tensor("woT", [DG, E], BF16, kind="ExternalInput")
    bq = nc.dram_tensor("bq", [DG], F32, kind="ExternalInput")
    bk = nc.dram_tensor("bk", [DG], F32, kind="ExternalInput")
    bv = nc.dram_tensor("bv", [DG], BF16, kind="ExternalInput")
    out = nc.dram_tensor("out", [S, E], BF16, kind="ExternalOutput")

    with tile.TileContext(nc) as tc:
        with (
            tc.tile_pool(name="persist", bufs=1) as pp,
            tc.tile_pool(name="xin", bufs=1) as xin,
            tc.tile_pool(name="epool", bufs=8) as epool,
            tc.tile_pool(name="opool", bufs=6) as opool,
            tc.tile_pool(name="small", bufs=2) as small,
            tc.tile_pool(name="ps_a", bufs=2, space="PSUM") as ps_a,
            tc.tile_pool(name="ps_e", bufs=2, space="PSUM") as ps_e,
            tc.tile_pool(name="ps_o", bufs=2, space="PSUM") as ps_o,
        ):
            # ---- SBUF homes for streamed inputs ----
            wq_sb = pp.tile([128, ET, DG], BF16, tag="wq")
            wk_sb = pp.tile([128, ET, DG], BF16, tag="wk")
            wv_sb = pp.tile([128, ET, DG], BF16, tag="wv")
            wo_sb = pp.tile([128, 2, E], BF16, tag="wo")
            bq_sb = pp.tile([128, 2], F32, tag="bq")
            bk_sb = pp.tile([128, 2], F32, tag="bk")
            bv_sb = pp.tile([1, DG], BF16, tag="bv")
            xq_sb = xin.tile([128, ET, S], BF16, tag="xq")
            xk_sb = xin.tile([128, ET, S], BF16, tag="xk")
            xv_sb = xin.tile([128, ET, S], BF16, tag="xv")

            # ---- DMA dispatch plan.  Queue facts (measured): only sync,
            # scalar and gpsimd can issue DMAs.  A sync/scalar dispatch
            # returns in ~0.8us when its queue is idle but blocks the engine
            # while a previous transfer is in flight; gpsimd's software queue
            # never blocks (descriptors are appended, ~1us engine time each).
            # The 16 DMA sub-engines cap aggregate HBM read at ~340GB/s,
            # shared roughly equally among active queues (~115GB/s each with
            # three active).  The c0 working set (wq|xq0 -> wk|xk0 -> wv|xv0)
            # is striped across all three queues in deadline order so the PE
            # ramps straight from warm-up into chunk-0 projections; sync (no
            # compute) then swallows all the late chunks and their blocking.
            xq_view = xqT.ap().rearrange("(a p) s -> p a s", p=128)
            xk_view = xkT.ap().rearrange("(a p) s -> p a s", p=128)
            xv_view = xvT.ap().rearrange("(a p) s -> p a s", p=128)

            def xchunk(eng, view, sb, c, half=None):
                lo, hi = (0, ET) if half is None else (4 * half, 4 * half + 4)
                eng.dma_start(
                    sb[:, lo:hi, SC * c:SC * (c + 1)],
                    view[:, lo:hi, SC * c:SC * (c + 1)])

            # warm-up stationary data: memset on vector so the PE's p-state
            # ramp does not wait on any gpsimd constant or DMA
            warmt = pp.tile([128, SC], BF16, tag="warmt")
            nc.vector.memset(warmt[:], 1.0)

            # Queue model (measured): a sync/scalar dispatch OCCUPIES the
            # issuing engine until its queue ring has a free slot (~2
            # descriptors deep); descriptors in a ring stream CONCURRENTLY,
            # splitting that queue's share of the ~340GB/s aggregate.
            # GpSimd's software queue costs ~1us of engine time per dispatch
            # and never blocks, and enqueues at engine-execution time -- so
            # mid-loop gpsimd dispatches are the one true pacing lever.
            # Plan: scalar gets exactly two early halves (engine free by
            # ~13us for exps); sync's engine (no compute) eats the late
            # pile; the first chunks are split in halves so both queues
            # land them in parallel.
            # c0 halves striped across sync+scalar so xq0 and xk0 land
            # TOGETHER as early as possible (the PE serializes their chains
            # anyway); sync's engine (no compute) then eats the late pile
            xchunk(nc.sync, xq_view, xq_sb, 0, half=0)
            xchunk(nc.scalar, xq_view, xq_sb, 0, half=1)
            xchunk(nc.sync, xk_view, xk_sb, 0, half=0)
            xchunk(nc.scalar, xk_view, xk_sb, 0, half=1)
            xchunk(nc.sync, xv_view, xv_sb, 0, half=0)
            xchunk(nc.sync, xk_view, xk_sb, 1, half=0)
            xchunk(nc.sync, xq_view, xq_sb, 2)
            xchunk(nc.sync, xv_view, xv_sb, 2)
            xchunk(nc.sync, xq_view, xq_sb, 3)

            nc.gpsimd.dma_start(wq_sb[:], wqT.ap().rearrange("(a p) d -> p a d", p=128))
            nc.gpsimd.dma_start(bq_sb[:], bq.ap().rearrange("(a p) -> p a", p=128))
            nc.gpsimd.dma_start(wk_sb[:], wkT.ap().rearrange("(a p) d -> p a d", p=128))
            nc.gpsimd.dma_start(bk_sb[:], bk.ap().rearrange("(a p) -> p a", p=128))

            ones_bf = pp.tile([1, 128], BF16, tag="ones_bf")
            nc.gpsimd.memset(ones_bf[:], 1.0)
            # causal strip: strip[p, f] = 1.0 if f - p >= 384 else 0.0
            strip = pp.tile([128, 896], BF16, tag="strip")
            nc.gpsimd.memset(strip[:], 1.0)
            nc.gpsimd.affine_select(
                out=strip[:],
                in_=strip[:],
                compare_op=mybir.AluOpType.is_ge,
                fill=0.0,
                base=-384,
                pattern=[[1, 896]],
                channel_multiplier=-1,
            )

            nc.gpsimd.dma_start(wv_sb[:], wvT.ap().rearrange("(a p) d -> p a d", p=128))
            nc.gpsimd.dma_start(bv_sb[:], bv.ap().rearrange("(o d) -> o d", o=1))
            xchunk(nc.gpsimd, xv_view, xv_sb, 0, half=1)
            xchunk(nc.gpsimd, xq_view, xq_sb, 1)
            xchunk(nc.gpsimd, xk_view, xk_sb, 1, half=1)
            xchunk(nc.gpsimd, xv_view, xv_sb, 1)
            nc.gpsimd.dma_start(wo_sb[:], woT.ap().rearrange("(a p) j -> p a j", p=128))

            # V store ones columns preset on the vector engine
            vst = [pp.tile([128, NH * (DH + 1)], BF16, tag=f"vst{st}", name=f"vst{st}")
                   for st in range(ST)]
            for st in range(ST):
                nc.vector.memset(vst[st][:], 1.0)

            def v_chain(st):
                ps = ps_a.tile([128, DG], F32, tag="ps_proj", name=f"pv{st}")
                nc.tensor.matmul(ps[:], ones_bf[:1, :], bv_sb[:1, :],
                                 start=True, stop=False)
                col = 128 * st
                for e in range(ET):
                    nc.tensor.matmul(
                        ps[:],
                        xv_sb[:, e, col:col + 128],
                        wv_sb[:, e, :],
                        start=False,
                        stop=(e == ET - 1),
                    )
                # single strided copy: psum [128, 4*64] -> vst cols {65h..65h+63}
                nc.vector.tensor_copy(
                    vst[st][:].rearrange("p (h x) -> p h x", h=NH)[:, :, 0:DH],
                    ps[:].rearrange("p (h x) -> p h x", h=NH),
                )

            # ---- QT/KT projection chains (emitted interleaved with attention) ----
            qt_sb = [pp.tile([128, S], BF16, tag=f"qt{d}", name=f"qt{d}") for d in range(2)]
            kt_sb = [pp.tile([128, S], BF16, tag=f"kt{d}", name=f"kt{d}") for d in range(2)]

            def proj_chain(x_sb, w_sb, b_sb, dst, sc, d):
                ps = ps_a.tile([128, SC], F32, tag="ps_proj",
                               name=f"pj{dst[0].name}{sc}{d}")
                for e in range(ET):
                    nc.tensor.matmul(
                        ps[:],
                        w_sb[:, e, 128 * d:128 * (d + 1)],
                        x_sb[:, e, SC * sc:SC * (sc + 1)],
                        start=(e == 0),
                        stop=(e == ET - 1),
                    )
                nc.vector.tensor_scalar_add(
                    dst[d][:, SC * sc:SC * (sc + 1)], ps[:], b_sb[:, d:d + 1]
                )

            def q_chain(sc, d):
                proj_chain(xq_sb, wq_sb, bq_sb, qt_sb, sc, d)

            def k_chain(sc, d):
                proj_chain(xk_sb, wk_sb, bk_sb, kt_sb, sc, d)

            # ---- phase 2 + 3: qc-major attention with output-projection bursts ----
            at_sb = [pp.tile([128, S], BF16, tag=f"at{d}", name=f"at{d}") for d in range(2)]

            def score_tile(pair, qc, t):
                """Both heads' score matmuls -> one 2-bank psum, single exp."""
                qt, kt = qt_sb[pair], kt_sb[pair]
                diag = t >= 4 * qc
                dd = 128 * t - SC * qc if diag else 0
                w = SC - dd
                pse = ps_e.tile([128, 2, SC], F32, tag="pse", name=f"pse{pair}{qc}{t}")
                for i in range(2):
                    p0 = 64 * i
                    nc.tensor.matmul(
                        pse[:, i, 0:w],
                        kt[p0:p0 + DH, 128 * t:128 * (t + 1)],
                        qt[p0:p0 + DH, SC * qc + dd:SC * (qc + 1)],
                        start=True, stop=True,
                    )
                e_sb = epool.tile([128, 2, SC], BF16, tag="esb",
                                  name=f"esb{pair}{qc}{t}")
                nc.scalar.activation(
                    e_sb[:, :, 0:w], pse[:, :, 0:w],
                    mybir.ActivationFunctionType.Exp,
                    bias=0.0, scale=float(SCALE),
                )
                if diag:
                    # only the first 128 trimmed columns straddle the triangle
                    for i in range(2):
                        nc.vector.tensor_mul(
                            e_sb[:, i, 0:128], e_sb[:, i, 0:128], strip[:, 384:512]
                        )
                return e_sb, dd, w

            def out_burst(qc, last=False, half=None):
                # output projection burst for finished qi rows of chunk qc
                sts = range(4 * qc, 4 * (qc + 1))
                if half is not None:
                    sts = sts[2 * half:2 * half + 2]
                for st in sts:
                    for jc in range(2):
                        ps = ps_a.tile([128, SC], F32, tag="ps_proj",
                                       name=f"psb3{st}{jc}")
                        for d in range(2):
                            nc.tensor.matmul(
                                ps[:],
                                at_sb[d][:, 128 * st:128 * (st + 1)],
                                wo_sb[:, d, SC * jc:SC * (jc + 1)],
                                start=(d == 0), stop=(d == 1),
                            )
                        o_sb = opool.tile([128, SC], BF16, tag="osb",
                                          name=f"osb{st}{jc}")
                        if last and (st + jc) % 2 == 0:
                            nc.scalar.copy(o_sb[:], ps[:])
                        else:
                            nc.vector.tensor_copy(o_sb[:], ps[:])
                        # mid-kernel bursts dispatch from gpsimd (software
                        # queue: never blocks; sync's queue is full of late
                        # input chunks and would block the engine for tens
                        # of us mid-dispatch)
                        eng = [nc.sync, nc.scalar, nc.gpsimd][(2 * st + jc) % 3] \
                            if last else nc.gpsimd
                        eng.dma_start(
                            out.ap()[128 * st:128 * (st + 1), SC * jc:SC * (jc + 1)],
                            o_sb[:],
                        )

            def warm(n, name):
                # consumer-less matmuls on constant data: keep the PE's
                # p-state ramped while it would otherwise idle on DMA/deps
                for w_ in range(n):
                    dps = ps_e.tile([128, 2, SC], F32, tag="pse",
                                    name=f"warm_{name}{w_}")
                    nc.tensor.matmul(
                        dps[:, 0, :], warmt[:, 0:128], warmt[:, 0:SC],
                        start=True, stop=True,
                    )

            # prologue: p-state ramp + the two chains pair-0 scores need.
            # Everything else (q01/k01, V chains, next-chunk projections)
            # is a filler inside the attention stream, scheduled per
            # (chunk, pair) to match DMA arrival order.
            warm_ps = []
            warm(16, "boot")
            q_chain(0, 0)
            # q01 needs only xq0/wq: it covers the PE while xk0/wk land
            # (arrival jitter puts them anywhere in ~17-30us); a few more
            # dep-free warms absorb the remaining xk0 jitter before k00
            q_chain(0, 1)
            warm(3, "kwait")
            k_chain(0, 0)

            def v_(st):
                return lambda: v_chain(st)

            def q_(sc, d):
                return lambda: q_chain(sc, d)

            def k_(sc, d):
                return lambda: k_chain(sc, d)

            # q01 and v0 are emitted right after pair 0's first two score
            # tiles, in DMA-arrival order (v0 must precede the first AV
            # matmul); v(4qc..4qc+3) for qc>=1 run as that chunk's own
            # pair-0 fillers, each ahead of its AV tile.
            # each pair's t=0 filler slot gets a ~3us projection chain when
            # one is available: it covers the previous pair's norm latency
            # and the exp pipeline refill at the boundary
            PRE = {(0, 0): [k_(0, 1), v_(0)]}
            FILL = {
                (0, 0): [v_(1), v_(2), v_(3)],
                (0, 1): [q_(1, 0), q_(1, 1), k_(1, 0), k_(1, 1)],
                (1, 0): [k_(2, 0), v_(4), v_(5), v_(6), v_(7)],
                (1, 1): [k_(2, 1), q_(2, 0), q_(2, 1)],
                (2, 0): [k_(3, 0), v_(8), v_(9), v_(10), v_(11)],
                (2, 1): [k_(3, 1), v_(12), v_(13), q_(3, 0)],
                (3, 0): [q_(3, 1), v_(14), v_(15)],
            }
            # remaining chunks enqueued from gpsimd mid-loop (after that
            # pair's norm broadcasts) so they can't compete with the c0/c1
            # transfers for early bandwidth
            GPACED = {
                (0, 1): [lambda: xchunk(nc.gpsimd, xk_view, xk_sb, 2),
                         lambda: xchunk(nc.gpsimd, xk_view, xk_sb, 3)],
                (1, 0): [lambda: xchunk(nc.gpsimd, xv_view, xv_sb, 3)],
            }
            for qc in range(NSC):
                nt = 4 * (qc + 1)  # ki tiles needed (causal)
                # chunk 0 has few slots for its fillers: double up
                per_slot = 2 if qc == 0 else 1
                for pair in range(2):
                    fillers = FILL.get((qc, pair), [])
                    psos = [ps_o.tile([128, SC], F32, tag="pso", name=f"pso{pair}{qc}{i}")
                            for i in range(2)]
                    # software pipeline: scores run two ki-tiles ahead of AV
                    es = {t0: score_tile(pair, qc, t0)
                          for t0 in range(min(2, nt))}
                    for fn in PRE.get((qc, pair), []):
                        fn()
                    for t in range(nt):
                        if t + 2 < nt:
                            es[t + 2] = score_tile(pair, qc, t + 2)
                        if (t in (2, 4) and pair == 0 and 0 < qc < NSC - 1):
                            # prev chunk's projection, split so the exp stream
                            # never drains during the burst
                            out_burst(qc - 1, half=t // 2 - 1)
                        elif t == 2 * (1 - pair) and qc == NSC - 1:
                            # last chunk: one burst half per pair (at t=0 for
                            # the filler-less pair 1, covering the boundary),
                            # so the final pairs keep PE work in hand while
                            # exps pace the AV stream
                            out_burst(qc - 1, half=pair)
                        elif (t >= 1 or (qc, pair) != (0, 0)) and fillers:
                            # for every pair after the first, t=0 is a filler
                            # slot too: a projection chain there covers the
                            # previous pair's norm-chain latency (the AV
                            # matmuls wait on the psum pool it holds)
                            for _ in range(per_slot):
                                if fillers:
                                    fillers.pop(0)()
                        e_sb, dd, w = es.pop(t)
                        for i in range(2):
                            nc.tensor.matmul(
                                psos[i][:65, dd:SC],
                                vst[t][:, 65 * (2 * pair + i):65 * (2 * pair + i + 1)],
                                e_sb[:, i, 0:w],
                                start=(t == 0), stop=(t == nt - 1),
                            )
                    # normalize by the ones-row sums: den copy + reciprocal on
                    # DVE (reciprocal can't read psum on hw), gpsimd partition
                    # broadcast, then multiply straight out of psum
                    recs, bcs = [], []
                    for i in range(2):
                        den = small.tile([1, SC], F32, tag="den", bufs=4,
                                         name=f"den{pair}{qc}{i}")
                        nc.vector.tensor_copy(den[:], psos[i][64:65, :])
                        rec = small.tile([1, SC], F32, tag="rec", bufs=4,
                                         name=f"rec{pair}{qc}{i}")
                        nc.vector.reciprocal_approx_fast(rec[:], den[:])
                        bc = small.tile([64, SC], F32, tag="bc", bufs=4,
                                        name=f"bc{pair}{qc}{i}")
                        nc.gpsimd.partition_broadcast(bc[:], rec[:1, :])
                        bcs.append(bc)
                    for i in range(2):
                        nc.vector.tensor_mul(
                            at_sb[pair][64 * i:64 * i + DH, SC * qc:SC * (qc + 1)],
                            psos[i][:64, :],
                            bcs[i][:],
                        )
                    if qc == NSC - 1 and pair == 1:
                        # keep the PE hot through the normalization chain so
                        # the final burst runs at full clock; prestart units
                        # 0-5's at0 accumulation (2 ps_a banks + 4 halves of
                        # 2 now-idle ps_e tiles) before the norm ops so their
                        # sem thresholds don't include the norm
                        # (emission-order counter semantics)
                        warm(12, "tail")
                        pse_t = None
                        for u in range(6):
                            st, jc = 12 + u // 2, u % 2
                            if u < 2:
                                pv = ps_a.tile([128, SC], F32, tag="ps_proj",
                                               name=f"psw{u}")[:]
                            else:
                                if u % 2 == 0:
                                    pse_t = ps_e.tile([128, 2, SC], F32,
                                                      tag="pse",
                                                      name=f"psw_e{u // 2}")
                                pv = pse_t[:, u % 2, :]
                            nc.tensor.matmul(
                                pv,
                                at_sb[0][:, 128 * st:128 * (st + 1)],
                                wo_sb[:, 0, SC * jc:SC * (jc + 1)],
                                start=True, stop=False,
                            )
                            warm_ps.append(pv)
                    for fn in GPACED.get((qc, pair), []):
                        fn()
                    assert not fillers, (qc, pair, len(fillers))

            # final burst: units 0-5 only need their at1 accumulation; the
            # last two run full chains on the ps_o banks the norm released
            for u in range(8):
                st, jc = 12 + u // 2, u % 2
                if u < 6:
                    ps = warm_ps[u]
                else:
                    ps = ps_o.tile([128, SC], F32, tag="pso", name=f"psf{u}")[:]
                    nc.tensor.matmul(
                        ps,
                        at_sb[0][:, 128 * st:128 * (st + 1)],
                        wo_sb[:, 0, SC * jc:SC * (jc + 1)],
                        start=True, stop=False,
                    )
                nc.tensor.matmul(
                    ps,
                    at_sb[1][:, 128 * st:128 * (st + 1)],
                    wo_sb[:, 1, SC * jc:SC * (jc + 1)],
                    start=False, stop=True,
                )
                o_sb = opool.tile([128, SC], BF16, tag="osb", name=f"osbf{u}")
                # final DMAs ride sync/scalar only: gpsimd's queue must be
                # empty at the end or its teardown DRAIN waits on it.  The
                # last four units split copy+DMA in halves across both
                # engines/queues to shorten the closing serial chain.
                if u < 4:
                    if u % 2 == 0:
                        nc.scalar.copy(o_sb[:], ps)
                    else:
                        nc.vector.tensor_copy(o_sb[:], ps)
                    eng = [nc.sync, nc.scalar][u % 2]
                    eng.dma_start(
                        out.ap()[128 * st:128 * (st + 1), SC * jc:SC * (jc + 1)],
                        o_sb[:],
                    )
                else:
                    h = SC // 2
                    nc.scalar.copy(o_sb[:, 0:h], ps[:, 0:h])
                    nc.vector.tensor_copy(o_sb[:, h:SC], ps[:, h:SC])
                    nc.sync.dma_start(
                        out.ap()[128 * st:128 * (st + 1),
                                 SC * jc:SC * jc + h],
                        o_sb[:, 0:h],
                    )
                    nc.scalar.dma_start(
                        out.ap()[128 * st:128 * (st + 1),
                                 SC * jc + h:SC * (jc + 1)],
                        o_sb[:, h:SC],
                    )

    nc.compile()
    return nc


def _get_nc():
    if "nc" not in _CACHED:
        _CACHED["nc"] = _build()
    return _CACHED["nc"]


def _in_maps(q, k, v, Wq, bq, Wk, bk, Wv, bv, Wo, bo):
    B = q.shape[0]
    f32 = np.float32
    xT = {}
    for b in range(B):
        xT[("q", b)] = np.ascontiguousarray(q[b].T).astype(NP_BF16)
        xT[("k", b)] = np.ascontiguousarray(k[b].T).astype(NP_BF16)
        xT[("v", b)] = np.ascontiguousarray(v[b].T).astype(NP_BF16)
    maps = []
    for c in range(8):
        b, g = c // 4, c % 4
        rows = slice(DG * g, DG * (g + 1))
        maps.append({
            "xqT": xT[("q", b)],
            "xkT": xT[("k", b)],
            "xvT": xT[("v", b)],
            "wqT": np.ascontiguousarray(Wq[rows, :].T).astype(NP_BF16),
            "wkT": np.ascontiguousarray(Wk[rows, :].T).astype(NP_BF16),
            "wvT": np.ascontiguousarray(Wv[rows, :].T).astype(NP_BF16),
            "woT": np.ascontiguousarray(Wo[:, rows].T).astype(NP_BF16),
            "bq": np.ascontiguousarray(bq[rows], dtype=f32),
            "bk": np.ascontiguousarray(bk[rows], dtype=f32),
            "bv": np.ascontiguousarray(bv[rows]).astype(NP_BF16),
        })
    return maps


def _run(inputs, trace=False):
    nc = _get_nc()
    maps = _in_maps(
        inputs["q"], inputs["k"], inputs["v"],
        inputs["Wq"], inputs["bq"], inputs["Wk"], inputs["bk"],
        inputs["Wv"], inputs["bv"], inputs["Wo"], inputs["bo"],
    )
    res = run_bass_kernel_spmd(nc, maps, list(range(8)), trace=trace)
    parts = [r["out"].astype(np.float32) for r in res.results]
    bo_row = np.asarray(inputs["bo"], dtype=np.float32)
    out = np.stack([
        parts[0] + parts[1] + parts[2] + parts[3] + bo_row,
        parts[4] + parts[5] + parts[6] + parts[7] + bo_row,
    ]).astype(np.float32)
    return out, res


def kernel(**inputs):
    out, _ = _run(inputs, trace=False)
    return out



# revision 51
# speedup vs baseline: 1.0173x; 1.0173x over previous
"""Multi-head attention (B=2, S=2048, D=1024, H=16, causal) on 8 trn2 cores.

Sharding: core c -> batch b = c//4, head group g = c%4 (4 heads of 64 dims):
data parallel over batch, tensor/head parallel within it (W_q/W_k/W_v split
column-wise, W_o row-wise per head group).  Each core computes Q/K/V
projections for its head group over the full sequence, causal flash-style
attention, and the partial output projection A_g @ Wo.T[g_rows, :].  The host
pre-transposes activations/weight slices to fp16, sums the 4 output partials
per batch (the row-parallel unshard), and adds bo.

Device dataflow (per core, fp16 matmuls, fp32 accumulation/softmax):
  inputs stream in 512-column chunks in first-use order so the PE never
           stalls on DMA and stays out of the low p-state
  QT, KT   [256, S] head-dim-major, computed by projection chains
           interleaved into the attention stream as PE gap fillers
  V_store  [S, 4*65] v columns + a ones column per head (so the AV matmul
           also produces the softmax denominators); the 16 V chains are
           themselves fillers (chunk qc's chains run early inside qc's own
           pair-0 stream, each one ahead of the AV tile that consumes it)
  scores^T tiles [128 ki, <=512 qi] -> exp on ACT (scale=1/sqrt(64)) -> E,
           diagonal tiles trimmed to their valid qi range; causal mask
           multiply only on the 128 straddling columns
  A^T      [256, S] accumulated in PSUM via (V|1)^T @ E, normalized by
           reciprocal of the ones-row read straight out of PSUM
           (gpsimd partition broadcast + fused multiply)
  out      [S, 1024] fp16 partial = A_g @ WoT_g, projected in per-chunk
           bursts slipped into the next chunk's attention stream

Scheduling (what got this from ~192us to ~163us):
  - DMA queues: only sync/scalar/gpsimd can issue DMAs.  A sync/scalar
    dispatch occupies the issuing engine until the queue ring (~2 deep)
    frees, and ring descriptors stream concurrently, splitting that
    queue's share of the ~340GB/s aggregate.  So the exp engine (scalar)
    gets exactly two early half-chunk transfers, sync's engine (no
    compute) eats the whole late pile, and gpsimd's never-blocking
    software queue carries weights + mid chunks, with the last chunks
    enqueued mid-loop (GPACED) so they can't steal early bandwidth.
  - The c0 halves are striped across sync+scalar so xq0 and xk0 land
    together (~16us); warm-up matmuls cover the DMA ramp and hold the
    PE's DVFS p-state up.
  - Every pair after the first gets a projection chain (or an out-
    projection burst half in the last chunk) in its t=0 filler slot to
    cover the previous pair's norm latency + exp pipeline refill.
  - The final burst prestarts 6 of 8 units' at0 accumulation in freed
    psum banks before the last norm, and the closing copies/DMAs are
    split across scalar+vector / sync+scalar to shorten the tail.
"""

import numpy as np

import concourse.bacc as bacc
import concourse.mybir as mybir
import concourse.tile as tile
from concourse.bass_utils import run_bass_kernel_spmd

F32 = mybir.dt.float32
BF16 = mybir.dt.float16  # fp16: same PE speed as bf16, 4x the mantissa
NP_BF16 = np.float16

S = 2048        # sequence length
E = 1024        # model dim (contraction for projections)
DG = 256        # head-group dim (4 heads x 64)
DH = 64         # head dim
NH = 4          # heads per core
ET = E // 128   # 8 e-tiles
ST = S // 128   # 16 s-tiles
SC = 512        # sequence chunk (psum free dim)
NSC = S // SC   # 4 chunks
SCALE = 1.0 / np.sqrt(DH)

_CACHED = {}


def _build():
    nc = bacc.Bacc("TRN2", target_bir_lowering=False, debug=False, num_devices=8)

    xqT = nc.dram_tensor("xqT", [E, S], BF16, kind="ExternalInput")
    xkT = nc.dram_tensor("xkT", [E, S], BF16, kind="ExternalInput")
    xvT = nc.dram_tensor("xvT", [E, S], BF16, kind="ExternalInput")
    wqT = nc.dram_tensor("wqT", [E, DG], BF16, kind="ExternalInput")
    wkT = nc.dram_tensor("wkT", [E, DG], BF16, kind="ExternalInput")
    wvT = nc.dram_tensor("wvT", [E, DG], BF16, kind="ExternalInput")
    woT = nc.dram_tensor("woT", [DG, E], BF16, kind="ExternalInput")
    bq = nc.dram_tensor("bq", [DG], F32, kind="ExternalInput")
    bk = nc.dram_tensor("bk", [DG], F32, kind="ExternalInput")
    bv = nc.dram_tensor("bv", [DG], BF16, kind="ExternalInput")
    out = nc.dram_tensor("out", [S, E], BF16, kind="ExternalOutput")

    with tile.TileContext(nc) as tc:
        with (
            tc.tile_pool(name="persist", bufs=1) as pp,
            tc.tile_pool(name="xin", bufs=1) as xin,
            tc.tile_pool(name="epool", bufs=8) as epool,
            tc.tile_pool(name="opool", bufs=6) as opool,
            tc.tile_pool(name="small", bufs=2) as small,
            tc.tile_pool(name="ps_a", bufs=2, space="PSUM") as ps_a,
            tc.tile_pool(name="ps_e", bufs=2, space="PSUM") as ps_e,
            tc.tile_pool(name="ps_o", bufs=2, space="PSUM") as ps_o,
        ):
            # ---- SBUF homes for streamed inputs ----
            wq_sb = pp.tile([128, ET, DG], BF16, tag="wq")
            wk_sb = pp.tile([128, ET, DG], BF16, tag="wk")
            wv_sb = pp.tile([128, ET, DG], BF16, tag="wv")
            wo_sb = pp.tile([128, 2, E], BF16, tag="wo")
            bq_sb = pp.tile([128, 2], F32, tag="bq")
            bk_sb = pp.tile([128, 2], F32, tag="bk")
            bv_sb = pp.tile([1, DG], BF16, tag="bv")
            xq_sb = xin.tile([128, ET, S], BF16, tag="xq")
            xk_sb = xin.tile([128, ET, S], BF16, tag="xk")
            xv_sb = xin.tile([128, ET, S], BF16, tag="xv")

            # ---- DMA dispatch plan.  Queue facts (measured): only sync,
            # scalar and gpsimd can issue DMAs.  A sync/scalar dispatch
            # returns in ~0.8us when its queue is idle but blocks the engine
            # while a previous transfer is in flight; gpsimd's software queue
            # never blocks (descriptors are appended, ~1us engine time each).
            # The 16 DMA sub-engines cap aggregate HBM read at ~340GB/s,
            # shared roughly equally among active queues (~115GB/s each with
            # three active).  The c0 working set (wq|xq0 -> wk|xk0 -> wv|xv0)
            # is striped across all three queues in deadline order so the PE
            # ramps straight from warm-up into chunk-0 projections; sync (no
            # compute) then swallows all the late chunks and their blocking.
            xq_view = xqT.ap().rearrange("(a p) s -> p a s", p=128)
            xk_view = xkT.ap().rearrange("(a p) s -> p a s", p=128)
            xv_view = xvT.ap().rearrange("(a p) s -> p a s", p=128)

            def xchunk(eng, view, sb, c, half=None):
                lo, hi = (0, ET) if half is None else (4 * half, 4 * half + 4)
                eng.dma_start(
                    sb[:, lo:hi, SC * c:SC * (c + 1)],
                    view[:, lo:hi, SC * c:SC * (c + 1)])

            # warm-up stationary data: memset on vector so the PE's p-state
            # ramp does not wait on any gpsimd constant or DMA
            warmt = pp.tile([128, SC], BF16, tag="warmt")
            nc.vector.memset(warmt[:], 1.0)

            # Queue model (measured): a sync/scalar dispatch OCCUPIES the
            # issuing engine until its queue ring has a free slot (~2
            # descriptors deep); descriptors in a ring stream CONCURRENTLY,
            # splitting that queue's share of the ~340GB/s aggregate.
            # GpSimd's software queue costs ~1us of engine time per dispatch
            # and never blocks, and enqueues at engine-execution time -- so
            # mid-loop gpsimd dispatches are the one true pacing lever.
            # Plan: scalar gets exactly two early halves (engine free by
            # ~13us for exps); sync's engine (no compute) eats the late
            # pile; the first chunks are split in halves so both queues
            # land them in parallel.
            # c0 halves striped across sync+scalar so xq0 and xk0 land
            # TOGETHER as early as possible (the PE serializes their chains
            # anyway); sync's engine (no compute) then eats the late pile
            xchunk(nc.sync, xq_view, xq_sb, 0, half=0)
            xchunk(nc.scalar, xq_view, xq_sb, 0, half=1)
            xchunk(nc.sync, xk_view, xk_sb, 0, half=0)
            xchunk(nc.scalar, xk_view, xk_sb, 0, half=1)
            xchunk(nc.sync, xv_view, xv_sb, 0, half=0)
            xchunk(nc.sync, xk_view, xk_sb, 1, half=0)
            xchunk(nc.sync, xq_view, xq_sb, 2)
            xchunk(nc.sync, xv_view, xv_sb, 2)
            xchunk(nc.sync, xq_view, xq_sb, 3)

            nc.gpsimd.dma_start(wq_sb[:], wqT.ap().rearrange("(a p) d -> p a d", p=128))
            nc.gpsimd.dma_start(bq_sb[:], bq.ap().rearrange("(a p) -> p a", p=128))
            nc.gpsimd.dma_start(wk_sb[:], wkT.ap().rearrange("(a p) d -> p a d", p=128))
            nc.gpsimd.dma_start(bk_sb[:], bk.ap().rearrange("(a p) -> p a", p=128))

            ones_bf = pp.tile([1, 128], BF16, tag="ones_bf")
            nc.gpsimd.memset(ones_bf[:], 1.0)
            # causal strip: strip[p, f] = 1.0 if f - p >= 384 else 0.0
            strip = pp.tile([128, 896], BF16, tag="strip")
            nc.gpsimd.memset(strip[:], 1.0)
            nc.gpsimd.affine_select(
                out=strip[:],
                in_=strip[:],
                compare_op=mybir.AluOpType.is_ge,
                fill=0.0,
                base=-384,
                pattern=[[1, 896]],
                channel_multiplier=-1,
            )

            nc.gpsimd.dma_start(wv_sb[:], wvT.ap().rearrange("(a p) d -> p a d", p=128))
            nc.gpsimd.dma_start(bv_sb[:], bv.ap().rearrange("(o d) -> o d", o=1))
            xchunk(nc.gpsimd, xv_view, xv_sb, 0, half=1)
            xchunk(nc.gpsimd, xq_view, xq_sb, 1)
            xchunk(nc.gpsimd, xk_view, xk_sb, 1, half=1)
            xchunk(nc.gpsimd, xv_view, xv_sb, 1)
            nc.gpsimd.dma_start(wo_sb[:], woT.ap().rearrange("(a p) j -> p a j", p=128))

            # V store ones columns preset on the vector engine
            vst = [pp.tile([128, NH * (DH + 1)], BF16, tag=f"vst{st}", name=f"vst{st}")
                   for st in range(ST)]
            for st in range(ST):
                nc.vector.memset(vst[st][:], 1.0)

            def v_chain(st):
                ps = ps_a.tile([128, DG], F32, tag="ps_proj", name=f"pv{st}")
                nc.tensor.matmul(ps[:], ones_bf[:1, :], bv_sb[:1, :],
                                 start=True, stop=False)
                col = 128 * st
                for e in range(ET):
                    nc.tensor.matmul(
                        ps[:],
                        xv_sb[:, e, col:col + 128],
                        wv_sb[:, e, :],
                        start=False,
                        stop=(e == ET - 1),
                    )
                # single strided copy: psum [128, 4*64] -> vst cols {65h..65h+63}
                nc.vector.tensor_copy(
                    vst[st][:].rearrange("p (h x) -> p h x", h=NH)[:, :, 0:DH],
                    ps[:].rearrange("p (h x) -> p h x", h=NH),
                )

            # ---- QT/KT projection chains (emitted interleaved with attention) ----
            qt_sb = [pp.tile([128, S], BF16, tag=f"qt{d}", name=f"qt{d}") for d in range(2)]
            kt_sb = [pp.tile([128, S], BF16, tag=f"kt{d}", name=f"kt{d}") for d in range(2)]

            def proj_chain(x_sb, w_sb, b_sb, dst, sc, d):
                ps = ps_a.tile([128, SC], F32, tag="ps_proj",
                               name=f"pj{dst[0].name}{sc}{d}")
                for e in range(ET):
                    nc.tensor.matmul(
                        ps[:],
                        w_sb[:, e, 128 * d:128 * (d + 1)],
                        x_sb[:, e, SC * sc:SC * (sc + 1)],
                        start=(e == 0),
                        stop=(e == ET - 1),
                    )
                nc.vector.tensor_scalar_add(
                    dst[d][:, SC * sc:SC * (sc + 1)], ps[:], b_sb[:, d:d + 1]
                )

            def q_chain(sc, d):
                proj_chain(xq_sb, wq_sb, bq_sb, qt_sb, sc, d)

            def k_chain(sc, d):
                proj_chain(xk_sb, wk_sb, bk_sb, kt_sb, sc, d)

            # ---- phase 2 + 3: qc-major attention with output-projection bursts ----
            at_sb = [pp.tile([128, S], BF16, tag=f"at{d}", name=f"at{d}") for d in range(2)]

            def score_tile(pair, qc, t):
                """Both heads' score matmuls -> one 2-bank psum, single exp."""
                qt, kt = qt_sb[pair], kt_sb[pair]
                diag = t >= 4 * qc
                dd = 128 * t - SC * qc if diag else 0
                w = SC - dd
                pse = ps_e.tile([128, 2, SC], F32, tag="pse", name=f"pse{pair}{qc}{t}")
                for i in range(2):
                    p0 = 64 * i
                    nc.tensor.matmul(
                        pse[:, i, 0:w],
                        kt[p0:p0 + DH, 128 * t:128 * (t + 1)],
                        qt[p0:p0 + DH, SC * qc + dd:SC * (qc + 1)],
                        start=True, stop=True,
                    )
                e_sb = epool.tile([128, 2, SC], BF16, tag="esb",
                                  name=f"esb{pair}{qc}{t}")
                nc.scalar.activation(
                    e_sb[:, :, 0:w], pse[:, :, 0:w],
                    mybir.ActivationFunctionType.Exp,
                    bias=0.0, scale=float(SCALE),
                )
                if diag:
                    # only the first 128 trimmed columns straddle the triangle
                    for i in range(2):
                        nc.vector.tensor_mul(
                            e_sb[:, i, 0:128], e_sb[:, i, 0:128], strip[:, 384:512]
                        )
                return e_sb, dd, w

            def out_burst(qc, last=False, half=None):
                # output projection burst for finished qi rows of chunk qc
                sts = range(4 * qc, 4 * (qc + 1))
                if half is not None:
                    sts = sts[2 * half:2 * half + 2]
                for st in sts:
                    for jc in range(2):
                        ps = ps_a.tile([128, SC], F32, tag="ps_proj",
                                       name=f"psb3{st}{jc}")
                        for d in range(2):
                            nc.tensor.matmul(
                                ps[:],
                                at_sb[d][:, 128 * st:128 * (st + 1)],
                                wo_sb[:, d, SC * jc:SC * (jc + 1)],
                                start=(d == 0), stop=(d == 1),
                            )
                        o_sb = opool.tile([128, SC], BF16, tag="osb",
                                          name=f"osb{st}{jc}")
                        if last and (st + jc) % 2 == 0:
                            nc.scalar.copy(o_sb[:], ps[:])
                        else:
                            nc.vector.tensor_copy(o_sb[:], ps[:])
                        # mid-kernel bursts dispatch from gpsimd (software
                        # queue: never blocks; sync's queue is full of late
                        # input chunks and would block the engine for tens
                        # of us mid-dispatch)
                        eng = [nc.sync, nc.scalar, nc.gpsimd][(2 * st + jc) % 3] \
                            if last else nc.gpsimd
                        eng.dma_start(
                            out.ap()[128 * st:128 * (st + 1), SC * jc:SC * (jc + 1)],
                            o_sb[:],
                        )

            def warm(n, name):
                # consumer-less matmuls on constant data: keep the PE's
                # p-state ramped while it would otherwise idle on DMA/deps
                for w_ in range(n):
                    dps = ps_e.tile([128, 2, SC], F32, tag="pse",
                                    name=f"warm_{name}{w_}")
                    nc.tensor.matmul(
                        dps[:, 0, :], warmt[:, 0:128], warmt[:, 0:SC],
                        start=True, stop=True,
                    )

            # prologue: p-state ramp + the two chains pair-0 scores need.
            # Everything else (q01/k01, V chains, next-chunk projections)
            # is a filler inside the attention stream, scheduled per
            # (chunk, pair) to match DMA arrival order.
            warm_ps = []
            warm(16, "boot")
            q_chain(0, 0)
            # q01 needs only xq0/wq: it covers the PE while xk0/wk land
            # (arrival jitter puts them anywhere in ~17-30us)
            q_chain(0, 1)
            k_chain(0, 0)

            def v_(st):
                return lambda: v_chain(st)

            def q_(sc, d):
                return lambda: q_chain(sc, d)

            def k_(sc, d):
                return lambda: k_chain(sc, d)

            # q01 and v0 are emitted right after pair 0's first two score
            # tiles, in DMA-arrival order (v0 must precede the first AV
            # matmul); v(4qc..4qc+3) for qc>=1 run as that chunk's own
            # pair-0 fillers, each ahead of its AV tile.
            # each pair's t=0 filler slot gets a ~3us projection chain when
            # one is available: it covers the previous pair's norm latency
            # and the exp pipeline refill at the boundary
            PRE = {(0, 0): [k_(0, 1), v_(0)]}
            FILL = {
                (0, 0): [v_(1), v_(2), v_(3)],
                (0, 1): [q_(1, 0), q_(1, 1), k_(1, 0), k_(1, 1)],
                (1, 0): [k_(2, 0), v_(4), v_(5), v_(6), v_(7)],
                (1, 1): [k_(2, 1), q_(2, 0), q_(2, 1)],
                (2, 0): [k_(3, 0), v_(8), v_(9), v_(10), v_(11)],
                (2, 1): [k_(3, 1), v_(12), v_(13), q_(3, 0)],
                (3, 0): [q_(3, 1), v_(14), v_(15)],
            }
            # remaining chunks enqueued from gpsimd mid-loop (after that
            # pair's norm broadcasts) so they can't compete with the c0/c1
            # transfers for early bandwidth
            GPACED = {
                (0, 1): [lambda: xchunk(nc.gpsimd, xk_view, xk_sb, 2),
                         lambda: xchunk(nc.gpsimd, xk_view, xk_sb, 3)],
                (1, 0): [lambda: xchunk(nc.gpsimd, xv_view, xv_sb, 3)],
            }
            for qc in range(NSC):
                nt = 4 * (qc + 1)  # ki tiles needed (causal)
                # chunk 0 has few slots for its fillers: double up
                per_slot = 2 if qc == 0 else 1
                for pair in range(2):
                    fillers = FILL.get((qc, pair), [])
                    psos = [ps_o.tile([128, SC], F32, tag="pso", name=f"pso{pair}{qc}{i}")
                            for i in range(2)]
                    # software pipeline: scores run two ki-tiles ahead of AV
                    es = {t0: score_tile(pair, qc, t0)
                          for t0 in range(min(2, nt))}
                    for fn in PRE.get((qc, pair), []):
                        fn()
                    for t in range(nt):
                        if t + 2 < nt:
                            es[t + 2] = score_tile(pair, qc, t + 2)
                        if (t in (2, 4) and pair == 0 and 0 < qc < NSC - 1):
                            # prev chunk's projection, split so the exp stream
                            # never drains during the burst
                            out_burst(qc - 1, half=t // 2 - 1)
                        elif t == 2 * (1 - pair) and qc == NSC - 1:
                            # last chunk: one burst half per pair (at t=0 for
                            # the filler-less pair 1, covering the boundary),
                            # so the final pairs keep PE work in hand while
                            # exps pace the AV stream
                            out_burst(qc - 1, half=pair)
                        elif (t >= 1 or (qc, pair) != (0, 0)) and fillers:
                            # for every pair after the first, t=0 is a filler
                            # slot too: a projection chain there covers the
                            # previous pair's norm-chain latency (the AV
                            # matmuls wait on the psum pool it holds)
                            for _ in range(per_slot):
                                if fillers:
                                    fillers.pop(0)()
                        e_sb, dd, w = es.pop(t)
                        for i in range(2):
                            nc.tensor.matmul(
                                psos[i][:65, dd:SC],
                                vst[t][:, 65 * (2 * pair + i):65 * (2 * pair + i + 1)],
                                e_sb[:, i, 0:w],
                                start=(t == 0), stop=(t == nt - 1),
                            )
                    # normalize by the ones-row sums: den copy + reciprocal on
                    # DVE (reciprocal can't read psum on hw), gpsimd partition
                    # broadcast, then multiply straight out of psum
                    recs, bcs = [], []
                    for i in range(2):
                        den = small.tile([1, SC], F32, tag="den", bufs=4,
                                         name=f"den{pair}{qc}{i}")
                        nc.vector.tensor_copy(den[:], psos[i][64:65, :])
                        rec = small.tile([1, SC], F32, tag="rec", bufs=4,
                                         name=f"rec{pair}{qc}{i}")
                        nc.vector.reciprocal_approx_fast(rec[:], den[:])
                        bc = small.tile([64, SC], F32, tag="bc", bufs=4,
                                        name=f"bc{pair}{qc}{i}")
                        nc.gpsimd.partition_broadcast(bc[:], rec[:1, :])
                        bcs.append(bc)
                    for i in range(2):
                        nc.vector.tensor_mul(
                            at_sb[pair][64 * i:64 * i + DH, SC * qc:SC * (qc + 1)],
                            psos[i][:64, :],
                            bcs[i][:],
                        )
                    if qc == NSC - 1 and pair == 1:
                        # keep the PE hot through the normalization chain so
                        # the final burst runs at full clock; prestart units
                        # 0-5's at0 accumulation (2 ps_a banks + 4 halves of
                        # 2 now-idle ps_e tiles) before the norm ops so their
                        # sem thresholds don't include the norm
                        # (emission-order counter semantics)
                        warm(12, "tail")
                        pse_t = None
                        for u in range(6):
                            st, jc = 12 + u // 2, u % 2
                            if u < 2:
                                pv = ps_a.tile([128, SC], F32, tag="ps_proj",
                                               name=f"psw{u}")[:]
                            else:
                                if u % 2 == 0:
                                    pse_t = ps_e.tile([128, 2, SC], F32,
                                                      tag="pse",
                                                      name=f"psw_e{u // 2}")
                                pv = pse_t[:, u % 2, :]
                            nc.tensor.matmul(
                                pv,
                                at_sb[0][:, 128 * st:128 * (st + 1)],
                                wo_sb[:, 0, SC * jc:SC * (jc + 1)],
                                start=True, stop=False,
                            )
                            warm_ps.append(pv)
                    for fn in GPACED.get((qc, pair), []):
                        fn()
                    assert not fillers, (qc, pair, len(fillers))

            # final burst: units 0-5 only need their at1 accumulation; the
            # last two run full chains on the ps_o banks the norm released
            for u in range(8):
                st, jc = 12 + u // 2, u % 2
                if u < 6:
                    ps = warm_ps[u]
                else:
                    ps = ps_o.tile([128, SC], F32, tag="pso", name=f"psf{u}")[:]
                    nc.tensor.matmul(
                        ps,
                        at_sb[0][:, 128 * st:128 * (st + 1)],
                        wo_sb[:, 0, SC * jc:SC * (jc + 1)],
                        start=True, stop=False,
                    )
                nc.tensor.matmul(
                    ps,
                    at_sb[1][:, 128 * st:128 * (st + 1)],
                    wo_sb[:, 1, SC * jc:SC * (jc + 1)],
                    start=False, stop=True,
                )
                o_sb = opool.tile([128, SC], BF16, tag="osb", name=f"osbf{u}")
                # final DMAs ride sync/scalar only: gpsimd's queue must be
                # empty at the end or its teardown DRAIN waits on it.  The
                # last four units split copy+DMA in halves across both
                # engines/queues to shorten the closing serial chain.
                if u < 4:
                    if u % 2 == 0:
                        nc.scalar.copy(o_sb[:], ps)
                    else:
                        nc.vector.tensor_copy(o_sb[:], ps)
                    eng = [nc.sync, nc.scalar][u % 2]
                    eng.dma_start(
                        out.ap()[128 * st:128 * (st + 1), SC * jc:SC * (jc + 1)],
                        o_sb[:],
                    )
                else:
                    h = SC // 2
                    nc.scalar.copy(o_sb[:, 0:h], ps[:, 0:h])
                    nc.vector.tensor_copy(o_sb[:, h:SC], ps[:, h:SC])
                    nc.sync.dma_start(
                        out.ap()[128 * st:128 * (st + 1),
                                 SC * jc:SC * jc + h],
                        o_sb[:, 0:h],
                    )
                    nc.scalar.dma_start(
                        out.ap()[128 * st:128 * (st + 1),
                                 SC * jc + h:SC * (jc + 1)],
                        o_sb[:, h:SC],
                    )

    nc.compile()
    return nc


def _get_nc():
    if "nc" not in _CACHED:
        _CACHED["nc"] = _build()
    return _CACHED["nc"]


def _in_maps(q, k, v, Wq, bq, Wk, bk, Wv, bv, Wo, bo):
    B = q.shape[0]
    f32 = np.float32
    xT = {}
    for b in range(B):
        xT[("q", b)] = np.ascontiguousarray(q[b].T).astype(NP_BF16)
        xT[("k", b)] = np.ascontiguousarray(k[b].T).astype(NP_BF16)
        xT[("v", b)] = np.ascontiguousarray(v[b].T).astype(NP_BF16)
    maps = []
    for c in range(8):
        b, g = c // 4, c % 4
        rows = slice(DG * g, DG * (g + 1))
        maps.append({
            "xqT": xT[("q", b)],
            "xkT": xT[("k", b)],
            "xvT": xT[("v", b)],
            "wqT": np.ascontiguousarray(Wq[rows, :].T).astype(NP_BF16),
            "wkT": np.ascontiguousarray(Wk[rows, :].T).astype(NP_BF16),
            "wvT": np.ascontiguousarray(Wv[rows, :].T).astype(NP_BF16),
            "woT": np.ascontiguousarray(Wo[:, rows].T).astype(NP_BF16),
            "bq": np.ascontiguousarray(bq[rows], dtype=f32),
            "bk": np.ascontiguousarray(bk[rows], dtype=f32),
            "bv": np.ascontiguousarray(bv[rows]).astype(NP_BF16),
        })
    return maps


def _run(inputs, trace=False):
    nc = _get_nc()
    maps = _in_maps(
        inputs["q"], inputs["k"], inputs["v"],
        inputs["Wq"], inputs["bq"], inputs["Wk"], inputs["bk"],
        inputs["Wv"], inputs["bv"], inputs["Wo"], inputs["bo"],
    )
    res = run_bass_kernel_spmd(nc, maps, list(range(8)), trace=trace)
    parts = [r["out"].astype(np.float32) for r in res.results]
    bo_row = np.asarray(inputs["bo"], dtype=np.float32)
    out = np.stack([
        parts[0] + parts[1] + parts[2] + parts[3] + bo_row,
        parts[4] + parts[5] + parts[6] + parts[7] + bo_row,
    ]).astype(np.float32)
    return out, res


def kernel(**inputs):
    out, _ = _run(inputs, trace=False)
    return out

